# revision 20
# baseline (speedup 1.0000x reference)
"""ColumnParallelLinearWithMoE Trainium2 kernel.

Expert-parallel: expert e -> NeuronCore e. Each core computes
    y_e [8192, 512] = x_e [8192, 1024] @ W_e.T [1024, 512] + b_e
where x_e = input_[idx_list[e]] flattened over (per, seq).

Routing gather/scatter and the x transpose (to put the contraction dim on
SBUF partitions) happen on the host; the device does the dense matmul.
"""

import sys

if "/opt/trn_rl_repo" not in sys.path:
    sys.path.insert(0, "/opt/trn_rl_repo")

import numpy as np

# Problem constants (hardcoded per harness contract).
E = 8
BS = 64
S = 1024
D = 1024
OPP = 512
P = 128
TOK = (BS // E) * S  # 8192 tokens per expert
KT = D // P          # 8 contraction tiles
TW = 512             # token-superblock width staged in SBUF
NSUP = TOK // TW
TPS = TW // P        # token tiles (of 128) per superblock

# Matmul dtype variant: "f32" (exact, ~466us), "f32r" (fp32 bits on the
# fast PE path, rel err ~1.5e-4, ~172us), "bf16" (host-cast inputs, fp32
# accumulate/output, rel err ~2.4e-3, ~134us).
VARIANT = "bf16"

_programs: dict[str, tuple] = {}


def _build(variant: str):
    import concourse.bacc as bacc
    import concourse.tile as tile
    from concourse import mybir

    if variant == "f32":
        mm_dt = mybir.dt.float32
        np_in = np.float32
    elif variant == "f32r":
        mm_dt = mybir.dt.float32r
        np_in = np.float32
    elif variant == "bf16":
        import ml_dtypes

        mm_dt = mybir.dt.bfloat16
        np_in = ml_dtypes.bfloat16
    else:
        raise ValueError(variant)

    nc = bacc.Bacc(None, target_bir_lowering=False, debug=False)

    xt = nc.dram_tensor("xt", [D, TOK], mm_dt, kind="ExternalInput")
    wt = nc.dram_tensor("wt", [D, OPP], mm_dt, kind="ExternalInput")
    bias = nc.dram_tensor("bias", [P, OPP], mybir.dt.float32, kind="ExternalInput")
    y = nc.dram_tensor("y", [TOK, OPP], mybir.dt.float32, kind="ExternalOutput")

    # Batched-DMA views: one dma_start per x/y superblock (split across all
    # 16 SDMA engines), so the issuing engine isn't the bottleneck.
    xt_r = xt.rearrange("(k p) t -> p k t", p=P)        # [128, KT, TOK]
    wt_r = wt.rearrange("(k p) c -> p k c", p=P)        # [128, KT, OPP]
    y_r = y.rearrange("(s j p) c -> p s j c", p=P, j=TPS)  # [128, NSUP, TPS, OPP]

    with tile.TileContext(nc) as tc:
        with (
            tc.tile_pool(name="wpool", bufs=1) as wpool,
            tc.tile_pool(name="bpool", bufs=1) as bpool,
            tc.tile_pool(name="xpool", bufs=4) as xpool,
            tc.tile_pool(name="opool", bufs=2) as opool,
            tc.tile_pool(name="pspool", bufs=4, space="PSUM") as pspool,
        ):
            # PE prewarm: dummy matmuls on a zeroed tile bridging the whole
            # initial-load window (~10us), so HAM un-throttles (1.2 -> 2.4
            # GHz) and stays warm until the first real matmul. Results are
            # never read.
            warm_src = wpool.tile([P, OPP], mybir.dt.bfloat16, tag="warm")
            nc.gpsimd.memset(warm_src[:], 0.0)
            warm_ps = pspool.tile([P, OPP], mybir.dt.float32, tag="warmps")
            for _ in range(6):
                nc.tensor.matmul(
                    warm_ps[:], warm_src[:, :P], warm_src[:], start=True, stop=True
                )

            # Ramp: the first token-tile's x chunk and w[k=0] land first so
            # the first accumulation group starts ~2us after the preamble;
            # everything else arrives while the PE chews. Later supers use
            # one big DMA each (HWDGE issue rate ~0.6us/dma_start caps early
            # bandwidth if transfers are small).
            x0_sb = xpool.tile([P, KT, TW], mm_dt, tag="x")
            w_sb = wpool.tile([P, KT, OPP], mm_dt)
            nc.sync.dma_start(out=x0_sb[:, :, 0:P], in_=xt_r[:, :, 0:P])
            nc.sync.dma_start(out=w_sb[:, 0, :], in_=wt_r[:, 0, :])
            nc.sync.dma_start(out=w_sb[:, 1:4, :], in_=wt_r[:, 1:4, :])
            nc.sync.dma_start(out=w_sb[:, 4:, :], in_=wt_r[:, 4:, :])

            # Bias right after w: the first PSUM eviction (tensor_add) needs
            # it ~13us in; a late bias holds PSUM banks and stalls group 5.
            bias_sb = bpool.tile([P, OPP], mybir.dt.float32)
            nc.sync.dma_start(out=bias_sb[:], in_=bias[:])

            for j in range(1, TPS):
                nc.sync.dma_start(
                    out=x0_sb[:, :, j * P : (j + 1) * P],
                    in_=xt_r[:, :, j * P : (j + 1) * P],
                )

            for s in range(NSUP):
                if s == 0:
                    x_sb = x0_sb
                elif s == 1:
                    # Super 1 races the tail of the preload; per-j chunks keep
                    # each PE gap under the 3.4us HAM re-throttle window.
                    x_sb = xpool.tile([P, KT, TW], mm_dt, tag="x")
                    for j in range(TPS):
                        nc.sync.dma_start(
                            out=x_sb[:, :, j * P : (j + 1) * P],
                            in_=xt_r[:, :, TW + j * P : TW + (j + 1) * P],
                        )
                else:
                    x_sb = xpool.tile([P, KT, TW], mm_dt, tag="x")
                    nc.sync.dma_start(
                        out=x_sb[:], in_=xt_r[:, :, s * TW : (s + 1) * TW]
                    )
                o_sb = opool.tile([P, TPS, OPP], mybir.dt.float32, tag="o")
                for j in range(TPS):
                    ps = pspool.tile([P, OPP], mybir.dt.float32)
                    for k in range(KT):
                        nc.tensor.matmul(
                            ps[:],
                            x_sb[:, k, j * P : (j + 1) * P],
                            w_sb[:, k, :],
                            start=(k == 0),
                            stop=(k == KT - 1),
                        )
                    nc.vector.tensor_add(o_sb[:, j, :], ps[:], bias_sb[:])
                # Stores go out on the scalar-engine HWDGE ring so they never
                # queue in front of the sync-ring loads. The final super's
                # store is split per token-tile so the tail drains sooner.
                if s < NSUP - 1:
                    nc.scalar.dma_start(out=y_r[:, s, :, :], in_=o_sb[:])
                else:
                    for j in range(TPS):
                        nc.scalar.dma_start(
                            out=y_r[:, s, j, :], in_=o_sb[:, j, :]
                        )

    nc.compile()
    return nc, np_in


def _get_program(variant: str):
    if variant not in _programs:
        _programs[variant] = _build(variant)
    return _programs[variant]


def kernel(input_, idx_list, W, b, **_ignored):
    from concourse.bass_utils import run_bass_kernel_spmd

    input_ = np.asarray(input_)
    idx = np.asarray(idx_list).astype(np.int64)
    W = np.asarray(W, dtype=np.float32)
    b = np.asarray(b, dtype=np.float32)

    nc, np_in = _get_program(VARIANT)

    in_maps = []
    for e in range(E):
        xg = input_[idx[e]].reshape(TOK, D).astype(np.float32, copy=False)
        xtr = np.ascontiguousarray(xg.T).astype(np_in)
        wtr = np.ascontiguousarray(W[e].T).astype(np_in)
        bias_bc = np.ascontiguousarray(
            np.broadcast_to(b[e][None, :], (P, OPP))
        ).astype(np.float32)
        in_maps.append({"xt": xtr, "wt": wtr, "bias": bias_bc})

    res = run_bass_kernel_spmd(nc, in_maps, core_ids=list(range(E)))

    out = np.zeros((BS, S, E * OPP), dtype=input_.dtype)
    for e in range(E):
        ye = np.asarray(res.results[e]["y"], dtype=input_.dtype)
        out[idx[e], :, e * OPP : (e + 1) * OPP] = ye.reshape(BS // E, S, OPP)
    return out
